# revision 14
# baseline (speedup 1.0000x reference)
"""MultiHeadAttnBlock Trainium2 kernel.

Computation (per batch element, one NeuronCore each — data-parallel over B=8):
  h   = GroupNorm(32, C)(x) * gn_w + gn_b
  qkv = qkv_w @ h + qkv_b          (1x1 conv == channel matmul)
  per head (8 heads, 64 ch):  w = softmax(q.k * ch**-0.5);  a = w @ v
  out = x + proj_w @ a + proj_b

Layout strategy (per core, x is [C=512, T=2048]):
  - h, q, k, a kept "natural" [channel(part), T(free)] as 4 tiles of [128, 2048]
  - v computed transposed: vT [T(part), vch(free)] tiles [128, 16, 8, 65]
    (65th column = ones -> softmax row-sums come free out of the apply matmul)
  - scores computed transposed: S[s, t] = sum_ch k[ch,s] q[ch,t] so that the
    apply matmul (contraction over s) has s on partitions.
  - softmax denominators: exp scores are multiplied against vT||ones, giving
    unnormalized a (rows 0:64) and row-sums (row 64) in one PSUM accumulator;
    normalization happens once per head on the [64, 2048] result.
  - exp is computed without max-subtraction: scores*ch**-0.5 are O(1) here
    (folded scale 0.125 applied inside the ACT exp instruction).
All matmuls run in bf16 (fp32 PSUM accumulate); everything else fp32.
"""

import sys

if "/opt/trn_rl_repo" not in sys.path:
    sys.path.insert(0, "/opt/trn_rl_repo")

import os

import ml_dtypes
import numpy as np

_DBG = os.environ.get("K_DEBUG", "")

import concourse.bacc as bacc
import concourse.bass as bass
import concourse.tile as tile
from concourse import mybir
from concourse.bass_utils import run_bass_kernel_spmd

F32 = mybir.dt.float32
BF16 = mybir.dt.bfloat16
AF = mybir.ActivationFunctionType
OP = mybir.AluOpType

B, C, T = 8, 512, 2048
G, NH, CHD = 32, 8, 64
NCI = C // 128   # 4 channel tiles
NT = T // 512    # 4 t-chunks per row tile
NSJ = T // 128   # 16 s-tiles
EPS = 1e-5
SCALE2 = float(CHD ** -0.5)  # folded q/k scale, applied inside exp


def _build():
    nc = bacc.Bacc("TRN2", target_bir_lowering=False, debug=False)

    x_d = nc.dram_tensor("x", [C, T], F32, kind="ExternalInput")
    wqk_d = nc.dram_tensor("wqk", [128, NCI, 1024], BF16, kind="ExternalInput")
    wv_d = nc.dram_tensor("wv", [128, NCI, C], BF16, kind="ExternalInput")
    wp_d = nc.dram_tensor("wp", [128, NCI, C], BF16, kind="ExternalInput")
    qkb_d = nc.dram_tensor("qkb", [128, 8], F32, kind="ExternalInput")
    vb_d = nc.dram_tensor("vb", [1, C], BF16, kind="ExternalInput")
    pb_d = nc.dram_tensor("pb", [128, NCI], F32, kind="ExternalInput")
    gnw_d = nc.dram_tensor("gnw", [128, NCI], F32, kind="ExternalInput")
    gnb_d = nc.dram_tensor("gnb", [128, NCI], F32, kind="ExternalInput")
    ind_d = nc.dram_tensor("ind", [128, 8], F32, kind="ExternalInput")
    indT_d = nc.dram_tensor("indT", [8, 128], F32, kind="ExternalInput")
    out_d = nc.dram_tensor("out", [C, T], F32, kind="ExternalOutput")

    with tile.TileContext(nc) as tc:
        with (
            tc.tile_pool(name="consts", bufs=1) as consts,
            tc.tile_pool(name="big", bufs=1) as big,
            tc.tile_pool(name="expp", bufs=6) as expp,
            tc.tile_pool(name="rbp", bufs=1) as rbp,
            tc.tile_pool(name="small", bufs=2) as small,
            tc.tile_pool(name="outp", bufs=2) as outp,
            tc.tile_pool(name="drp", bufs=2, space="DRAM") as drp,
        ):
            # ---- constants ----
            wqk = consts.tile([128, NCI, 1024], BF16)
            nc.sync.dma_start(wqk[:], wqk_d[:])
            wv = consts.tile([128, NCI, C], BF16)
            nc.sync.dma_start(wv[:], wv_d[:])
            wp = consts.tile([128, NCI, C], BF16)
            nc.sync.dma_start(wp[:], wp_d[:])
            qkb = consts.tile([128, 8], F32)
            nc.sync.dma_start(qkb[:], qkb_d[:])
            vb = consts.tile([1, C], BF16)
            nc.sync.dma_start(vb[:], vb_d[:])
            pb = consts.tile([128, NCI], F32)
            nc.sync.dma_start(pb[:], pb_d[:])
            gnw = consts.tile([128, NCI], F32)
            nc.sync.dma_start(gnw[:], gnw_d[:])
            gnb = consts.tile([128, NCI], F32)
            nc.sync.dma_start(gnb[:], gnb_d[:])
            ind = consts.tile([128, 8], F32)
            nc.sync.dma_start(ind[:], ind_d[:])
            indT = consts.tile([8, 128], F32)
            nc.sync.dma_start(indT[:], indT_d[:])
            ones1 = consts.tile([1, 128], BF16)
            nc.vector.memset(ones1[:], 1.0)
            eps8 = consts.tile([8, 1], F32)
            nc.vector.memset(eps8[:], EPS)

            # ---- big persistent tensors ----
            x_sb = big.tile([128, NCI, T], F32)
            h_sb = big.tile([128, NCI, T], BF16)
            q_sb = big.tile([128, NCI, T], BF16)
            k_sb = big.tile([128, NCI, T], BF16)
            a_sb = big.tile([128, NCI, T], BF16)
            vT_sb = big.tile([128, NSJ, NH, CHD + 1], BF16)
            nc.vector.memset(vT_sb[:, :, :, CHD], 1.0)

            # ---- phase G: load x + GroupNorm stats ----
            stats_pc = big.tile([128, NCI, 2], F32)  # per-channel (sum, sumsq)
            sq_scr = big.tile([128, T], BF16)        # scratch for sumsq pass

            for ci in range(NCI):
                nc.sync.dma_start(
                    x_sb[:, ci, :], x_d[ci * 128 : (ci + 1) * 128, :]
                )
            for ci in range(NCI):
                nc.vector.reduce_sum(
                    stats_pc[:, ci, 0:1], x_sb[:, ci, :], axis=mybir.AxisListType.X
                )
                nc.vector.tensor_mul(sq_scr[:], x_sb[:, ci, :], x_sb[:, ci, :])
                nc.vector.reduce_sum(
                    stats_pc[:, ci, 1:2], sq_scr[:], axis=mybir.AxisListType.X
                )

            with tc.tile_pool(name="psq", bufs=4, space="PSUM") as psq:
                # group-sum the per-channel stats (16 adjacent channels/group)
                gs = psq.tile([8, NCI, 2], F32, tag="ps")
                for ci in range(NCI):
                    nc.tensor.matmul(gs[:, ci, :], ind[:], stats_pc[:, ci, :])
                mv = small.tile([8, NCI, 2], F32)
                nc.vector.tensor_scalar_mul(mv[:], gs[:], 1.0 / (16 * T))
                var = small.tile([8, NCI], F32)
                nc.vector.tensor_mul(var[:], mv[:, :, 0], mv[:, :, 0])
                nc.vector.tensor_sub(var[:], mv[:, :, 1], var[:])
                sd = small.tile([8, NCI], F32)
                nc.scalar.activation(sd[:], var[:], AF.Sqrt, bias=eps8[:])
                rstd = small.tile([8, NCI], F32)
                nc.vector.reciprocal(rstd[:], sd[:])
                # one Newton step for rsqrt accuracy: y*(1.5 - 0.5*(var+eps)*y^2)
                ve = small.tile([8, NCI], F32)
                nc.vector.tensor_scalar_add(ve[:], var[:], EPS)
                y2 = small.tile([8, NCI], F32)
                nc.vector.tensor_mul(y2[:], rstd[:], rstd[:])
                nc.vector.tensor_mul(y2[:], ve[:], y2[:])
                nc.vector.tensor_scalar(
                    out=y2[:], in0=y2[:], scalar1=-0.5, scalar2=1.5,
                    op0=OP.mult, op1=OP.add,
                )
                nc.vector.tensor_mul(rstd[:], rstd[:], y2[:])

                rmu = small.tile([8, NCI, 2], F32)
                nc.vector.tensor_copy(rmu[:, :, 0], rstd[:])
                nc.vector.tensor_copy(rmu[:, :, 1], mv[:, :, 0])
                bc = psq.tile([128, NCI, 2], F32, tag="ps")
                for ci in range(NCI):
                    nc.tensor.matmul(bc[:, ci, :], indT[:], rmu[:, ci, :])
                scale_c = small.tile([128, NCI], F32)
                nc.vector.tensor_mul(scale_c[:], bc[:, :, 0], gnw[:])
                bias_c = small.tile([128, NCI], F32)
                nc.vector.tensor_mul(bias_c[:], bc[:, :, 1], scale_c[:])
                nc.vector.tensor_sub(bias_c[:], gnb[:], bias_c[:])

                for ci in range(NCI):
                    nc.scalar.activation(
                        h_sb[:, ci, :], x_sb[:, ci, :], AF.Identity,
                        bias=bias_c[:, ci : ci + 1], scale=scale_c[:, ci : ci + 1],
                    )

                # ---- phase Q: q, k (natural layout) ----
                for m in range(8):
                    dst = q_sb if m < 4 else k_sb
                    for n in range(NT):
                        pq = psq.tile([128, 512], F32, tag="ps")
                        for ci in range(NCI):
                            nc.tensor.matmul(
                                pq[:],
                                wqk[:, ci, m * 128 : (m + 1) * 128],
                                h_sb[:, ci, n * 512 : (n + 1) * 512],
                                start=(ci == 0),
                                stop=(ci == NCI - 1),
                            )
                        nc.vector.tensor_scalar_add(
                            dst[:, m % 4, n * 512 : (n + 1) * 512],
                            pq[:],
                            qkb[:, m : m + 1],
                        )
                # ---- vT (transposed v) ----
                for tj in range(NSJ):
                    pv = psq.tile([128, 512], F32, tag="ps")
                    for ci in range(NCI):
                        nc.tensor.matmul(
                            pv[:],
                            h_sb[:, ci, tj * 128 : (tj + 1) * 128],
                            wv[:, ci, :],
                            start=(ci == 0),
                            stop=False,
                        )
                    nc.tensor.matmul(pv[:], ones1[:], vb[:], start=False, stop=True)
                    nc.vector.tensor_copy(
                        vT_sb[:, tj, :, 0:CHD],
                        pv[:].rearrange("p (h c) -> p h c", h=NH),
                    )

            # ---- phase A: attention, one head at a time ----
            with (
                tc.tile_pool(name="pss", bufs=2, space="PSUM") as pss,
                tc.tile_pool(name="psa", bufs=1, space="PSUM") as psa,
            ):
                for hh in range(NH if "stop_q" not in _DBG else 0):
                    m = hh // 2
                    p0 = 64 * (hh % 2)
                    acc = psa.tile([CHD + 1, T], F32, tag="acc")
                    for j in range(NSJ):
                        for th in range(2):
                            s_ps = pss.tile([128, 1024], F32, tag="sc")
                            for w in range(2):
                                o = th * 1024 + w * 512
                                nc.tensor.matmul(
                                    s_ps[:, w * 512 : (w + 1) * 512],
                                    k_sb[p0 : p0 + 64, m, j * 128 : (j + 1) * 128],
                                    q_sb[p0 : p0 + 64, m, o : o + 512],
                                )
                            e_sb = expp.tile([128, 1024], BF16)
                            nc.scalar.activation(e_sb[:], s_ps[:], AF.Exp, scale=SCALE2)
                            for w in range(2):
                                o = th * 1024 + w * 512
                                nc.tensor.matmul(
                                    acc[:, o : o + 512],
                                    vT_sb[:, j, hh, :],
                                    e_sb[:, w * 512 : (w + 1) * 512],
                                    start=(j == 0),
                                    stop=(j == NSJ - 1),
                                )
                    if "nonorm" in _DBG:
                        nc.vector.tensor_copy(a_sb[p0 : p0 + 64, m, :], acc[0:CHD, :])
                        continue
                    # normalize: a = acc[0:64] / acc[64]
                    rsum = small.tile([1, T], F32)
                    nc.vector.tensor_copy(rsum[:], acc[CHD : CHD + 1, :])
                    r16 = small.tile([128, 16], F32)
                    nc.sync.dma_start(r16[:], rsum[:])
                    ri16 = small.tile([128, 16], F32)
                    nc.vector.reciprocal(ri16[:], r16[:])
                    rd = drp.tile([1, T], F32)
                    nc.sync.dma_start(rd[:], ri16[:])
                    rb = rbp.tile([CHD, T], F32)
                    rd_ap = rd[:]
                    nc.sync.dma_start(
                        rb[:],
                        bass.AP(
                            tensor=rd_ap.tensor,
                            offset=rd_ap.offset,
                            ap=[[0, CHD]] + list(rd_ap.ap[1:]),
                        ),
                    )
                    nc.vector.tensor_mul(a_sb[p0 : p0 + 64, m, :], acc[0:CHD, :], rb[:])

            if "stop_q" in _DBG:
                # debug: bypass attention+proj, out = q
                for m in range(NCI):
                    om = outp.tile([128, T], F32)
                    nc.scalar.activation(om[:], q_sb[:, m, :], AF.Copy)
                    nc.sync.dma_start(out_d[m * 128 : (m + 1) * 128, :], om[:])

            # ---- phase P: proj + bias + residual ----
            with tc.tile_pool(name="psp", bufs=4, space="PSUM") as psp:
                for m in range(NCI if "stop_q" not in _DBG else 0):
                    om = outp.tile([128, T], F32)
                    for n in range(NT):
                        pp = psp.tile([128, 512], F32, tag="pp")
                        for ci in range(NCI):
                            nc.tensor.matmul(
                                pp[:],
                                wp[:, ci, m * 128 : (m + 1) * 128],
                                a_sb[:, ci, n * 512 : (n + 1) * 512],
                                start=(ci == 0),
                                stop=(ci == NCI - 1),
                            )
                        nc.vector.scalar_tensor_tensor(
                            out=om[:, n * 512 : (n + 1) * 512],
                            in0=pp[:],
                            scalar=pb[:, m : m + 1],
                            in1=x_sb[:, m, n * 512 : (n + 1) * 512],
                            op0=OP.add,
                            op1=OP.add,
                        )
                    nc.sync.dma_start(out_d[m * 128 : (m + 1) * 128, :], om[:])

    nc.compile()
    return nc


_NC = None


def _get_nc():
    global _NC
    if _NC is None:
        _NC = _build()
    return _NC


def _prep_inputs(x, gn_w, gn_b, qkv_w, qkv_b, proj_w, proj_b):
    bf = ml_dtypes.bfloat16
    f32 = np.float32
    x = np.asarray(x, dtype=f32)
    gn_w = np.asarray(gn_w, dtype=f32)
    gn_b = np.asarray(gn_b, dtype=f32)
    qkv_w = np.asarray(qkv_w, dtype=f32)
    qkv_b = np.asarray(qkv_b, dtype=f32)
    proj_w = np.asarray(proj_w, dtype=f32)
    proj_b = np.asarray(proj_b, dtype=f32)

    def wt(w):  # [O, C] -> [128, NCI, O] (lhsT tiles, bf16)
        o = w.shape[0]
        return np.ascontiguousarray(
            w.T.astype(bf).reshape(NCI, 128, o).transpose(1, 0, 2)
        )

    ind = np.zeros((128, 8), dtype=f32)
    ind[np.arange(128), np.arange(128) // 16] = 1.0

    common = {
        "wqk": wt(qkv_w[:1024]),
        "wv": wt(qkv_w[1024:]),
        "wp": wt(proj_w),
        "qkb": np.ascontiguousarray(qkv_b[:1024].reshape(8, 128).T),
        "vb": np.ascontiguousarray(qkv_b[1024:].astype(bf).reshape(1, C)),
        "pb": np.ascontiguousarray(proj_b.reshape(NCI, 128).T),
        "gnw": np.ascontiguousarray(gn_w.reshape(NCI, 128).T),
        "gnb": np.ascontiguousarray(gn_b.reshape(NCI, 128).T),
        "ind": ind,
        "indT": np.ascontiguousarray(ind.T),
    }
    in_maps = [dict(common, x=np.ascontiguousarray(x[b])) for b in range(B)]
    return in_maps


def kernel(x, gn_w, gn_b, qkv_w, qkv_b, proj_w, proj_b):
    nc = _get_nc()
    in_maps = _prep_inputs(x, gn_w, gn_b, qkv_w, qkv_b, proj_w, proj_b)
    res = run_bass_kernel_spmd(nc, in_maps, core_ids=list(range(B)))
    return np.stack([res.results[b]["out"] for b in range(B)]).astype(np.float32)
